# revision 3
# baseline (speedup 1.0000x reference)
"""BitNetV3Attention (B=2, S=2048, H=16, DH=128, D=2048) on 8 TRN2 NeuronCores.

Strategy (tensor-parallel over heads + row-parallel o_proj):
  - Each core owns 2 of 16 heads. It computes Q^T/K^T (head-transposed,
    [DH, B*S]) and V ([B*S, DH]) for its heads from the full hidden states
    (replicated read), runs causal flash-style attention per (head, batch),
    producing normalized attn_out^T slices [256, B*S].
  - Two AllToAll collectives (one per local head slot) redistribute attn_out
    from head-sharded to sequence-sharded: core j ends with
    attn_out^T[:, rows_j] for ALL 2048 model dims, where rows_j are 512 rows
    of the [4096, 2048] token matrix. The first A2A fires after local head 0
    finishes and overlaps head 1's attention.
  - Each core computes its 512 output rows against the full Wo (no
    all-reduce needed; outputs concatenate on host). o_proj accumulates
    even d-tiles (from A2A#0) before odd ones (A2A#1) so it can start
    before the second collective lands. Wo slabs prefetch during attention.

v2 changes vs baseline:
  - All matmul operands in bf16 (halves ht DMA; removes the fp32r 4x
    penalty on <256-wide diagonal tiles; transposes at 1.0 cyc/row).
  - DMAs split across both HWDGE queues: weights/consts/Wo on the ACT
    queue (nc.scalar), ht slabs / a2a / at / out on the SP queue (nc.sync)
    — kills the ~30us head-of-line startup stall.
  - Weight loads k-split (k0 | k1-3 | k4-15) and the first ht slab split
    (k0-1 | k2-7) so the first matmul starts ~1us in.
  - Attention t-loop software-pipelined 2 deep: scores matmul ps(t) issues
    before po/pd(t-2), so PE stays fed while DVE/ACT produce ex(t).
  - Wo slabs for ne=0,1 prefetch before attention.

Softmax skips max-subtraction (scores are O(5), exp is safe); the padding
mask rides the ScalarE activation's per-partition bias; the causal mask is
a single [128, 1024] additive template sliced per diagonal tile.
"""
import sys
for _p in ('/opt/trn_rl_repo', '/root/.axon_site/_ro/trn_rl_repo'):
    if _p not in sys.path:
        sys.path.append(_p)

import numpy as np

import concourse.mybir as mybir
import concourse.tile as tile
from concourse import bacc, bass_utils

B, S, H, DH = 2, 2048, 16, 128
D = H * DH                  # 2048
NS = B * S                  # 4096
NC = 8                      # cores
HL = H // NC                # 2 local heads
DSL = HL * DH               # 256 (d-slice per core)
ROWS = NS // NC             # 512 output rows per core
SCALE = 1.0 / float(np.sqrt(DH))
F32 = mybir.dt.float32
F32R = mybir.dt.float32r
BF16 = mybir.dt.bfloat16
MM_DT = BF16
EXP = mybir.ActivationFunctionType.Exp
NEG = -1.0e30

N_K = D // 128              # 16 contraction tiles
N_SC = NS // 512            # 8 s-chunks for QKV
N_QC = S // 512             # 4 q-chunks per batch


def build_bass(repeat=1, do_attn=True, do_a2a=True, do_oproj=True,
               only_a2a=False, n_colls=2, coll_kind="a2a"):
    nc = bacc.Bacc("TRN2", target_bir_lowering=False, debug=False, num_devices=NC)

    ht = nc.dram_tensor("ht", [D, NS], MM_DT, kind="ExternalInput").ap()
    wqt = nc.dram_tensor("wqt", [D, DSL], MM_DT, kind="ExternalInput").ap()
    wkt = nc.dram_tensor("wkt", [D, DSL], MM_DT, kind="ExternalInput").ap()
    wvt = nc.dram_tensor("wvt", [D, DSL], MM_DT, kind="ExternalInput").ap()
    wot = nc.dram_tensor("wot", [D, D], BF16, kind="ExternalInput").ap()
    pad = nc.dram_tensor("pad", [B, S], F32, kind="ExternalInput").ap()
    tri = nc.dram_tensor("tri", [128, 1024], F32, kind="ExternalInput").ap()
    onesd = nc.dram_tensor("ones", [128, 128], MM_DT, kind="ExternalInput").ap()
    idend = nc.dram_tensor("iden", [128, 128], MM_DT, kind="ExternalInput").ap()
    out = nc.dram_tensor("out", [ROWS, D], F32, kind="ExternalOutput").ap()

    with tile.TileContext(nc) as tc:
        with tc.tile_pool(name="dram", bufs=1, space="DRAM") as dram, \
             tc.tile_pool(name="const", bufs=1) as cpool:
            a2a_in = [dram.tile([NC, DH, 512], BF16, name=f"a2a_in{h}") for h in range(HL)]
            a2a_out = [dram.tile([NC, DH, 512], BF16, name=f"a2a_out{h}") for h in range(HL)]

            tri_sb = cpool.tile([128, 1024], F32)
            pad_sb = cpool.tile([128, B * 16], F32)
            ones_sb = cpool.tile([128, 128], MM_DT)
            iden_sb = cpool.tile([128, 128], MM_DT)

            def emit_consts():
                # On the ACT queue, sequenced by the caller between the small
                # early weight tiles and the bulk loads.
                nc.scalar.dma_start(iden_sb[:], idend)
                nc.scalar.dma_start(ones_sb[:], onesd)
                nc.scalar.dma_start(
                    pad_sb[:].rearrange("p (b t) -> p b t", b=B),
                    pad.rearrange("b (t p) -> p b t", p=128),
                )

            def emit_tri():
                # 512KB, first needed ~190us in at attention: keep it off the
                # bandwidth-bound startup window.
                nc.scalar.dma_start(tri_sb[:], tri)

            ag_out = None
            if only_a2a and coll_kind in ("ag", "barrier"):
                shp = ([NC, NC, DH, 512] if coll_kind == "ag"
                       else [NC, 128])
                ag_out = [
                    nc.dram_tensor(f"ag_out{h}", shp, BF16, kind="Internal",
                                   addr_space="Shared").ap()
                    for h in range(HL)]
                ag_in = [
                    nc.dram_tensor(f"ag_in{h}", shp[1:], BF16,
                                   kind="Internal").ap()
                    for h in range(HL)]
            for _rep in range(repeat):
                if only_a2a:
                    for h in range(min(HL, n_colls)):
                        if coll_kind == "a2a":
                            nc.gpsimd.collective_compute(
                                "AllToAll", mybir.AluOpType.bypass,
                                replica_groups=[list(range(NC))],
                                ins=[a2a_in[h].opt()], outs=[a2a_out[h].opt()])
                        else:
                            nc.gpsimd.collective_compute(
                                "AllGather", mybir.AluOpType.bypass,
                                replica_groups=[list(range(NC))],
                                ins=[ag_in[h].opt()], outs=[ag_out[h].opt()])
                    continue
                _emit_body(nc, tc, a2a_in, a2a_out, tri_sb, pad_sb, ones_sb,
                           iden_sb, ht, wqt, wkt, wvt, wot, out,
                           emit_consts=(emit_consts if _rep == 0 else None),
                           emit_tri=(emit_tri if _rep == 0 else None),
                           do_attn=do_attn, do_a2a=do_a2a, do_oproj=do_oproj)
    nc.compile()
    return nc


def _emit_qkv(nc, tc, qt_sb, kt_sb, v_sb, iden_sb, ht, wqt, wkt, wvt,
              emit_consts, emit_tri):
    with tc.tile_pool(name="wts", bufs=1) as wpool, \
         tc.tile_pool(name="hts", bufs=3) as hpool, \
         tc.tile_pool(name="vtt", bufs=2) as vpool, \
         tc.tile_pool(name="ps1", bufs=1, space="PSUM") as pp1:
        srcs = (("q", wqt), ("k", wkt), ("v", wvt))
        w_sb = {nm: wpool.tile([128, N_K * DSL], MM_DT, name=f"w{nm}")
                for nm, _ in srcs}
        # ACT queue, smallest-first so the k=0..3 matmuls start ~1us in.
        for nm, src in srcs:
            nc.scalar.dma_start(w_sb[nm][:, 0:DSL], src[0:128, :])
        for nm, src in srcs:
            nc.scalar.dma_start(
                w_sb[nm][:, DSL:4 * DSL].rearrange("p (t m) -> p t m", t=3),
                src[128:512, :].rearrange("(t p) m -> p t m", p=128),
            )
        if emit_consts is not None:
            emit_consts()
        for nm, src in srcs:
            nc.scalar.dma_start(
                w_sb[nm][:, 4 * DSL:].rearrange("p (t m) -> p t m", t=N_K - 4),
                src[512:, :].rearrange("(t p) m -> p t m", p=128),
            )
        if emit_tri is not None:
            emit_tri()

        ht_r = ht.rearrange("(k p) s -> p k s", p=128)
        # Early slabs arrive in k-granular pieces so the first matmuls never
        # wait on a whole-2MB transfer.
        splits = {(0, 0): (2, 3, 3), (0, 1): (4, 4), (1, 0): (4, 4), (1, 1): (4, 4)}
        for sc in range(N_SC):
            psq = [pp1.tile([128, 512], F32, tag=f"pq{h}", name=f"pq{h}") for h in range(HL)]
            psk = [pp1.tile([128, 512], F32, tag=f"pk{h}", name=f"pk{h}") for h in range(HL)]
            psvt = [pp1.tile([128, 512], F32, tag=f"pvt{h}", name=f"pvt{h}") for h in range(HL)]
            slabs = []
            for half in range(2):
                slab = hpool.tile([128, 8 * 512], MM_DT, tag="ht", name="htslab")
                view = slab[:].rearrange("p (k s) -> p k s", k=8)
                src = ht_r[:, 8*half:8*half+8, 512*sc:512*sc+512]
                k0 = 0
                for w in splits.get((sc, half), (8,)):
                    nc.sync.dma_start(view[:, k0:k0+w], src[:, k0:k0+w])
                    k0 += w
                slabs.append(slab)
            for k in range(N_K):
                htt = slabs[k // 8][:, 512*(k % 8):512*(k % 8)+512]
                fl = dict(start=(k == 0), stop=(k == N_K - 1))
                for h in range(HL):
                    nc.tensor.matmul(
                        psq[h][:], w_sb["q"][:, DSL*k+128*h:DSL*k+128*h+128],
                        htt, **fl)
                    nc.tensor.matmul(
                        psk[h][:], w_sb["k"][:, DSL*k+128*h:DSL*k+128*h+128],
                        htt, **fl)
                    nc.tensor.matmul(
                        psvt[h][:], w_sb["v"][:, DSL*k+128*h:DSL*k+128*h+128],
                        htt, **fl)
            # drain PSUM -> SBUF, split across DVE and ACT. The last chunk
            # drains via ACT only, keeping DVE clear for the first diagonal
            # mask-adds of attention (which gate the first exps).
            last = sc == N_SC - 1
            vtt = []
            for h in range(HL):
                if last:
                    nc.scalar.copy(
                        qt_sb[h][:, 512*sc:512*sc+512], psq[h][:])
                else:
                    nc.vector.tensor_copy(
                        qt_sb[h][:, 512*sc:512*sc+512], psq[h][:])
                nc.scalar.copy(
                    kt_sb[h][:, 512*sc:512*sc+512], psk[h][:])
                vt = vpool.tile([128, 512], MM_DT, tag=f"vtt{h}", name=f"vtt{h}")
                if h == 0 and not last:
                    nc.vector.tensor_copy(vt[:], psvt[h][:])
                else:
                    nc.scalar.copy(vt[:], psvt[h][:])
                vtt.append(vt)
            # PE-transpose V chunk to natural [s, dh] layout
            for h in range(HL):
                for m in range(4):
                    ptp = pp1.tile([128, 128], MM_DT, tag="ptp", name="ptp", bufs=2)
                    nc.tensor.transpose(
                        ptp[:], vtt[h][:, 128*m:128*m+128], iden_sb[:])
                    st = 4 * sc + m
                    if (h + m) % 2 == 0 and not last:
                        nc.vector.tensor_copy(
                            v_sb[h][:, 128*st:128*st+128], ptp[:])
                    else:
                        nc.scalar.copy(
                            v_sb[h][:, 128*st:128*st+128], ptp[:])


def _emit_attention(nc, tc, qt_sb, kt_sb, v_sb, tri_sb, pad_sb, ones_sb,
                    a2a_in, a2a_out, do_a2a, at_sb):
    # One flat software-pipelined stream over every (h, b, qc, t) tile:
    # scores matmuls run DEPTH tiles ahead of the po/pd consumers, across
    # chunk boundaries, so PE never drains while ACT produces ex.
    DEPTH = 4
    with tc.tile_pool(name="att", bufs=1) as apool, \
         tc.tile_pool(name="ps2", bufs=1, space="PSUM") as pp2:
        stream = [(h, b, qc, t)
                  for h in range(HL) for b in range(B) for qc in range(N_QC)
                  for t in range(4 * qc + 4)]
        state = {}   # (h,b,qc) -> [po, pd, {t: (ex, o)}]

        def emit_ps(h, b, qc, t):
            q0 = 512 * qc
            # columns sq < o are fully causal-masked; skip them
            o = max(0, 128 * t - q0)
            ps = pp2.tile([128, 512], F32, tag="ps", bufs=4, name="ps")
            nc.tensor.matmul(
                ps[:, o:512],
                kt_sb[h][:, S*b+128*t:S*b+128*t+128],
                qt_sb[h][:, S*b+q0+o:S*b+q0+512],
                start=True, stop=True)
            if t >= 4 * qc:
                # diagonal block: the causal template is nonzero only inside
                # the 128-wide band [o, o+128) (cols >= o+128 have q > k for
                # every k in this block) — add just the band, not [o:512].
                nc.vector.tensor_add(
                    ps[:, o:o+128], ps[:, o:o+128], tri_sb[:, 512:640])
            ex = apool.tile([128, 512], MM_DT, tag="ex", bufs=6, name="ex")
            nc.scalar.activation(
                ex[:, o:512], ps[:, o:512], EXP,
                bias=pad_sb[:, 16*b+t:16*b+t+1], scale=SCALE)
            if (h, b, qc) not in state:
                state[(h, b, qc)] = [None, None, {}]
            state[(h, b, qc)][2][t] = (ex, o)

        def emit_povd(h, b, qc, t):
            st_ = state[(h, b, qc)]
            if st_[0] is None:
                st_[0] = pp2.tile([128, 512], F32, tag="po", bufs=2, name="po")
                st_[1] = pp2.tile([128, 512], F32, tag="pd", bufs=2, name="pd")
            po, pd = st_[0], st_[1]
            ex, o = st_[2].pop(t)
            n_sk = 4 * qc + 4
            fl = dict(start=(t == 0), stop=(t == n_sk - 1))
            st = 16 * b + t
            nc.tensor.matmul(
                po[:, o:512], v_sb[h][:, 128*st:128*st+128],
                ex[:, o:512], **fl)
            nc.tensor.matmul(
                pd[:, o:512], ones_sb[:], ex[:, o:512], **fl)
            if t != n_sk - 1:
                return
            # chunk done: normalize and ship to the A2A staging buffer
            del state[(h, b, qc)]
            rec = apool.tile([128, 512], F32, tag="rec", bufs=2, name="rec")
            nc.vector.reciprocal(rec[:], pd[:])
            ao = apool.tile([128, 512], BF16, tag="ao", bufs=2, name="ao")
            nc.vector.tensor_mul(ao[:], po[:], rec[:])
            nc.sync.dma_start(a2a_in[h][4*b+qc, :, :], ao[:])
            if b == B - 1 and qc == N_QC - 1:
                if do_a2a:
                    # AllToAll for this head-slot (overlaps next head's attn)
                    nc.gpsimd.collective_compute(
                        "AllToAll", mybir.AluOpType.bypass,
                        replica_groups=[list(range(NC))],
                        ins=[a2a_in[h].opt()], outs=[a2a_out[h].opt()])
                # gather this slot's redistributed rows as soon as they land
                nc.sync.dma_start(
                    at_sb[h][:].rearrange("p (j s) -> p j s", j=8),
                    a2a_out[h].rearrange("j p s -> p j s"))

        for i, key in enumerate(stream):
            emit_ps(*key)
            if i >= DEPTH:
                emit_povd(*stream[i - DEPTH])
        for key in stream[len(stream) - DEPTH:]:
            emit_povd(*key)


def _emit_oproj(nc, tc, wopool, obpool, a2a_out, wot, out, wo_pre, at_sb,
                load_at):
    # Two passes over (ne, m): pass 0 accumulates the even d-tiles (head
    # slot 0, delivered by A2A#0) into SBUF partials — ~27us of PE work that
    # runs while A2A#1 is still in flight. Pass 1 adds the odd d-tiles and
    # ships the output rows.
    with tc.tile_pool(name="ps4", bufs=4, space="PSUM") as pp4, \
         tc.tile_pool(name="oacc", bufs=1) as accpool:
        if load_at:                    # half 0 = even g (head slot 0)
            for half in range(2):
                nc.sync.dma_start(
                    at_sb[half][:].rearrange("p (j s) -> p j s", j=8),
                    a2a_out[half].rearrange("j p s -> p j s"))
        wot_r2 = wot.rearrange("(t2 two p) e -> p two t2 e", p=128, two=2)
        acc = {}
        for half in range(2):
            for ne in range(4):
                pre = wo_pre.get((half, ne))
                slab = pre if pre is not None else _wo_slab(
                    nc, wopool, wot_r2, half, ne)
                for m in range(4):
                    pout = pp4.tile([128, 512], F32, tag="pout", name="pout",
                                    bufs=6)
                    for i in range(N_K // 2):
                        j = i        # g = 2*j + half lives at a2a slot half
                        nc.tensor.matmul(
                            pout[:],
                            at_sb[half][:, 512*j+128*m:512*j+128*m+128],
                            slab[:, 512*j:512*j+512],
                            start=(i == 0), stop=(i == N_K // 2 - 1))
                    if half == 0:
                        a = accpool.tile([128, 512], BF16, name=f"acc{ne}_{m}")
                        if (ne + m) % 2 == 0:
                            nc.vector.tensor_copy(a[:], pout[:])
                        else:
                            nc.scalar.copy(a[:], pout[:])
                        acc[(ne, m)] = a
                    else:
                        ob = obpool.tile([128, 512], F32, tag="ob", name="ob",
                                         bufs=4)
                        nc.vector.tensor_add(ob[:], pout[:], acc[(ne, m)][:])
                        nc.sync.dma_start(
                            out[128*m:128*m+128, 512*ne:512*ne+512], ob[:])


def _wo_slab(nc, wopool, wot_r2, half, ne):
    sl = wopool.tile([128, 8 * 512], BF16, tag=f"wo{half}",
                     name=f"wo{half}", bufs=2)
    nc.sync.dma_start(
        sl[:].rearrange("p (t e) -> p t e", t=8),
        wot_r2[:, half, :, 512*ne:512*ne+512])
    return sl


def _emit_body(nc, tc, a2a_in, a2a_out, tri_sb, pad_sb, ones_sb,
               iden_sb, ht, wqt, wkt, wvt, wot, out, emit_consts=None,
               emit_tri=None, do_attn=True, do_a2a=True, do_oproj=True):
    with tc.tile_pool(name="store", bufs=1) as spool:
        qt_sb = [spool.tile([128, NS], MM_DT, name=f"qt{h}") for h in range(HL)]
        kt_sb = [spool.tile([128, NS], MM_DT, name=f"kt{h}") for h in range(HL)]
        v_sb = [spool.tile([128, NS], MM_DT, name=f"v{h}") for h in range(HL)]

        _emit_qkv(nc, tc, qt_sb, kt_sb, v_sb, iden_sb, ht, wqt, wkt, wvt,
                  emit_consts, emit_tri)

        # o_proj pools open before attention so Wo slab DMAs can prefetch
        # into the space vacated by the QKV weight/ht pools during attention.
        with tc.tile_pool(name="oproj", bufs=1) as opool, \
             tc.tile_pool(name="wo", bufs=2) as wopool, \
             tc.tile_pool(name="ob", bufs=3) as obpool:
            wot_r2 = wot.rearrange("(t2 two p) e -> p two t2 e", p=128, two=2)
            # prefetch pass-0's first two Wo slabs (and pass-1's first) so
            # o_proj starts without waiting on weight DMA
            wo_pre = {(0, 0): _wo_slab(nc, wopool, wot_r2, 0, 0),
                      (0, 1): _wo_slab(nc, wopool, wot_r2, 0, 1),
                      (1, 0): _wo_slab(nc, wopool, wot_r2, 1, 0)}
            at_sb = [opool.tile([128, 8 * 512], BF16, name=f"at{half}")
                     for half in range(2)]
            if do_attn:
                _emit_attention(nc, tc, qt_sb, kt_sb, v_sb, tri_sb, pad_sb,
                                ones_sb, a2a_in, a2a_out, do_a2a, at_sb)
            if do_oproj:
                _emit_oproj(nc, tc, wopool, obpool, a2a_out, wot, out,
                            wo_pre, at_sb, load_at=not do_attn)


_NC_CACHE = None


def _get_nc():
    global _NC_CACHE
    if _NC_CACHE is None:
        _NC_CACHE = build_bass()
    return _NC_CACHE


def make_in_maps(hidden_states, attention_mask, Wq, Wk, Wv, Wo):
    import ml_dtypes
    mm_np = np.float32 if MM_DT == F32R else ml_dtypes.bfloat16
    x = np.ascontiguousarray(np.asarray(hidden_states, dtype=np.float32)).reshape(NS, D)
    ht = np.ascontiguousarray(x.T).astype(mm_np)                     # [D, NS]
    wqt = np.ascontiguousarray(np.asarray(Wq, dtype=np.float32).T).astype(mm_np)
    wkt = np.ascontiguousarray(np.asarray(Wk, dtype=np.float32).T).astype(mm_np)
    wvt = np.ascontiguousarray(np.asarray(Wv, dtype=np.float32).T).astype(mm_np)
    wot = np.ascontiguousarray(
        np.asarray(Wo, dtype=np.float32).T).astype(ml_dtypes.bfloat16)
    mask = np.asarray(attention_mask)
    pad = np.where(mask == 0, np.float32(NEG), np.float32(0.0)).astype(np.float32)
    tri = np.where(
        np.arange(1024, dtype=np.int64)[None, :] >= np.arange(128, dtype=np.int64)[:, None] + 512,
        np.float32(0.0), np.float32(NEG)).astype(np.float32)
    ones = np.ones((128, 128), dtype=np.float32)
    iden = np.eye(128, dtype=np.float32)

    in_maps = []
    for c in range(NC):
        sl = slice(DSL * c, DSL * c + DSL)
        in_maps.append({
            "ht": ht,
            "wqt": np.ascontiguousarray(wqt[:, sl]),
            "wkt": np.ascontiguousarray(wkt[:, sl]),
            "wvt": np.ascontiguousarray(wvt[:, sl]),
            "wot": wot,
            "pad": pad,
            "tri": tri,
            "ones": ones.astype(mm_np),
            "iden": iden.astype(mm_np),
        })
    return in_maps


def assemble_output(results):
    rows = np.concatenate([results[c]["out"] for c in range(NC)], axis=0)
    return rows.reshape(B, S, D).astype(np.float32)


def kernel(hidden_states, attention_mask, Wq, Wk, Wv, Wo):
    nc = _get_nc()
    in_maps = make_in_maps(hidden_states, attention_mask, Wq, Wk, Wv, Wo)
    res = bass_utils.run_bass_kernel_spmd(nc, in_maps, core_ids=list(range(NC)))
    return assemble_output(res.results)

